# revision 31
# baseline (speedup 1.0000x reference)
"""Trainium2 Bass kernel: top-2 MoE routing (E=16, D=H=2048), 8 NeuronCores.

Strategy (memory-regime optimal: only the 2 selected experts' weights are
ever read from HBM, in bf16):
  * Weights are sharded across cores *within* each expert: core c owns rows
    [c*256, (c+1)*256) of every expert's W1 and the matching contraction
    slice of W2, both pre-swizzled on the host to the exact SBUF layout
    ([128, free]) and cast to bf16, so each dynamic load is one
    per-partition-contiguous 1MB DMA.
  * Every core redundantly computes the gating on-device from one packed
    (x | Wg.T | one | bg) DMA: logits via PE matvec, then top-2 value+index
    in two DVE ops (max / max_index).  The two expert indices go to SP
    registers, which drive the four dynamic weight DMAs.
  * b1 rides as two extra bf16 columns of each W1 slice; b2/8 is injected
    into the output PSUM accumulation by a [16x128]^T @ gvec matmul where
    gvec[e] = tkg_k for the selected experts (computed via a tiny K=1
    transpose matmul), so the post-stream tail is one PSUM->SBUF copy and
    one 8KB store.
  * tkg_k = sigmoid(+/-(v0-v1)) folded through tanh (single ACT table);
    the reference's +1e-6 softmax-sum regularizer is dropped (relative
    effect ~1e-5, far below the 2e-2 gate).
  * Each core's gate-weighted partial output is returned per-core; the
    host sums the 8 partials (free) and de-swizzles.
"""

import numpy as np

try:  # make concourse importable in bare environments
    import concourse.bacc  # noqa: F401
except ImportError:  # pragma: no cover
    import sys

    sys.path.insert(0, "/opt/trn_rl_repo")

import ml_dtypes

BF16 = ml_dtypes.bfloat16

E, D, H = 16, 2048, 2048
NCORES = 8
P = 128
RS = H // NCORES  # 256 rows of each expert held per core
NCH = RS // P  # 2 partition-chunks per 256 rows
DC = D // P  # 16 contraction chunks for layer 1
OC = H // P  # 16 output chunks for layer 2

PRER = 192  # rows of every expert's W1 (dc=0) prefetched speculatively
W1DYN = DC * RS - PRER  # 3904: dynamically-loaded W1 columns
W1W = W1DYN + NCH  # 3906: + b1 slice folded as 2 extra cols
W2W = NCH * H  # 4096
W2E1_SPLIT = True  # split W2e1 into oc0-13/oc14-15 loads (see comment below)
PKW = DC + DC * E + 1 + E  # 289: x | Wg.T | one | bg
COL_ONE = DC + DC * E  # 272
COL_BG = COL_ONE + 1  # 273
B2W = H + P  # 2176: b2/8 | ones_row (partition 0)
COL_ONES = H  # 2048

_BUILT = None


def _build(stage=2):
    """Build + compile the Bass program once. stage kept for test.py compat."""
    global _BUILT
    if _BUILT is not None and _BUILT[2] == stage:
        return _BUILT[:2]

    import concourse.bacc as bacc
    import concourse.bass as bass
    import concourse.tile as tile
    from concourse import mybir

    f32 = mybir.dt.float32
    bf16 = mybir.dt.bfloat16
    u32 = mybir.dt.uint32
    OP = mybir.AluOpType
    AF = mybir.ActivationFunctionType

    nc = bacc.Bacc(
        "TRN2", target_bir_lowering=False, debug=False, num_devices=NCORES
    )

    # ----- I/O ------------------------------------------------------------
    pack_d = nc.dram_tensor("pack", [P, PKW], bf16, kind="ExternalInput")
    b2pk_d = nc.dram_tensor("b2pk", [E, B2W], bf16, kind="ExternalInput")
    w1pre_d = nc.dram_tensor("w1pre", [P, PRER * E], bf16, kind="ExternalInput")
    w1s_d = nc.dram_tensor("w1s", [E, P, W1W], bf16, kind="ExternalInput")
    w2s_d = nc.dram_tensor("w2s", [E, P, W2W], bf16, kind="ExternalInput")
    out_d = nc.dram_tensor("out", [1, H], f32, kind="ExternalOutput")

    in_names = ["pack", "b2pk", "w1pre", "w1s", "w2s"]

    with tile.TileContext(nc) as tc:
        with (
            tc.tile_pool(name="sb", bufs=1) as sb,
            tc.tile_pool(name="ps", bufs=1, space="PSUM") as ps,
        ):
            # ----- static loads -------------------------------------------
            pk = sb.tile([P, PKW], bf16, tag="pk")
            nc.sync.dma_start(pk[:], pack_d.ap())
            # speculative prefetch: first PRER rows (dc=0) of every expert's
            # W1, streamed during the otherwise-idle gating window.  Layout
            # [p, r*E+e] so PE can matvec all 16 experts' partial h.
            w1pr = sb.tile([P, PRER * E], bf16, tag="w1pr")
            nc.sync.dma_start(w1pr[:], w1pre_d.ap())
            # b2pk rides the SP ring after the prefetch so it cannot steal
            # the prefetch's HWDGE slot (its data isn't needed until ~6us)
            b2pk = sb.tile([E, B2W], bf16, tag="b2pk")
            nc.sync.dma_start(b2pk[:], b2pk_d.ap())

            # ----- gating on PE: logits = Wg @ x (+ bg) -------------------
            lg = ps.tile([1, E], f32, tag="lg")
            for dc in range(DC):
                nc.tensor.matmul(
                    out=lg[:],
                    lhsT=pk[:, dc : dc + 1],
                    rhs=pk[:, DC + dc * E : DC + (dc + 1) * E],
                    start=(dc == 0),
                    stop=False,
                )
            nc.tensor.matmul(
                out=lg[:],
                lhsT=pk[0:1, COL_ONE : COL_ONE + 1],
                rhs=pk[0:1, COL_BG : COL_BG + E],
                start=False,
                stop=True,
            )

            # ----- top-2 value+index in two DVE ops -----------------------
            vals8 = sb.tile([1, 8], f32, tag="vals8")
            idx8 = sb.tile([1, 8], u32, tag="idx8")
            nc.vector.max(vals8[:], lg[:])
            nc.vector.max_index(idx8[:], vals8[:], lg[:])

            # ----- expert indices -> SP registers -> weight DMAs ----------
            # W2 of the 2nd expert is split so the final transfer is small
            # (oc 14..15 only): most layer-2 matmuls clear before the last
            # bytes land, shrinking the post-stream tail.
            OSP = OC - 2  # oc split point
            sv = []
            w1t = [
                sb.tile([P, W1W], bf16, tag=f"w1t{k}", name=f"w1t{k}")
                for k in range(2)
            ]
            w2t0 = sb.tile([P, W2W], bf16, tag="w2t0")
            if W2E1_SPLIT:
                w2t1a = sb.tile([P, NCH * OSP * P], bf16, tag="w2t1a")
                w2t1b = sb.tile([P, NCH * (OC - OSP) * P], bf16, tag="w2t1b")
            else:
                w2t1 = sb.tile([P, W2W], bf16, tag="w2t1")
            for k in range(2):
                reg = nc.sync.alloc_register(f"idx_sp{k}")
                nc.sync.reg_load(reg, idx8[0:1, k : k + 1])
                sv.append(nc.snap(reg, donate=True, min_val=0, max_val=E - 1))
                nc.sync.dma_start(
                    w1t[k][:],
                    w1s_d.ap()[bass.ds(sv[k], 1), :, :].rearrange(
                        "a p f -> p (a f)"
                    ),
                )
            nc.sync.dma_start(
                w2t0[:],
                w2s_d.ap()[bass.ds(sv[0], 1), :, :].rearrange("a p f -> p (a f)"),
            )
            w2e1 = w2s_d.ap()[bass.ds(sv[1], 1), :, :].rearrange(
                "a p (ic o) -> p (a ic) o", ic=NCH
            )
            if W2E1_SPLIT:
                nc.sync.dma_start(
                    w2t1a[:].rearrange("p (ic o) -> p ic o", ic=NCH),
                    w2e1[:, :, 0 : OSP * P],
                )
                nc.sync.dma_start(
                    w2t1b[:].rearrange("p (ic o) -> p ic o", ic=NCH),
                    w2e1[:, :, OSP * P : H],
                )
            else:
                # single W2e1 load keeps the HWDGE-DMA count at 8, so the
                # out-DMA's completion lane is checked LAST by the exit
                # drain (lane order = allocation order, 8 lanes round-robin)
                nc.sync.dma_start(
                    w2t1[:],
                    w2s_d.ap()[bass.ds(sv[1], 1), :, :].rearrange(
                        "a p f -> p (a f)"
                    ),
                )

            def w2_lhsT(k, oc, ic):
                if k == 0:
                    return w2t0[:, ic * H + oc * P : ic * H + (oc + 1) * P]
                if not W2E1_SPLIT:
                    return w2t1[:, ic * H + oc * P : ic * H + (oc + 1) * P]
                if oc < OSP:
                    w = OSP * P
                    return w2t1a[:, ic * w + oc * P : ic * w + (oc + 1) * P]
                w = (OC - OSP) * P
                o = oc - OSP
                return w2t1b[:, ic * w + o * P : ic * w + (o + 1) * P]

            # ----- normalized top-2 gates: tkg1 = sigmoid(v1-v0) ----------
            # sigma(z) = 0.5*tanh(z/2) + 0.5 keeps everything on the tanh
            # ACT table (no exp table load on the critical path).
            d01 = sb.tile([1, 1], f32, tag="d01")
            nc.vector.tensor_tensor(
                out=d01[:], in0=vals8[:, 1:2], in1=vals8[:, 0:1], op=OP.subtract
            )
            th01 = sb.tile([1, 1], f32, tag="th01")
            nc.scalar.activation(th01[:], d01[:], AF.Tanh, scale=0.5)
            tkg = sb.tile([1, 2], f32, tag="tkg")
            nc.vector.tensor_scalar(tkg[:, 1:2], th01[:], 0.5, 0.5, OP.mult, OP.add)
            nc.vector.tensor_scalar(
                tkg[:, 0:1], th01[:], -0.5, 0.5, OP.mult, OP.add
            )
            tkgb = sb.tile([1, 2], bf16, tag="tkgb")
            nc.vector.tensor_copy(tkgb[:], tkg[:])

            # gvf[e] = tkg_k if e == e_k else 0, in free layout:
            #   m2 = (lg >= v1) marks both, m1 = (lg >= v0) marks top-1
            #   gvf = tkg1*m2 + (tkg0-tkg1)*m1
            m1f = sb.tile([1, E], bf16, tag="m1f")
            m2f = sb.tile([1, E], bf16, tag="m2f")
            nc.vector.tensor_scalar(m2f[:], lg[:], vals8[:, 1:2], None, OP.is_ge)
            nc.vector.tensor_scalar(m1f[:], lg[:], vals8[:, 0:1], None, OP.is_ge)
            dt01 = sb.tile([1, 1], f32, tag="dt01")
            nc.vector.tensor_tensor(
                out=dt01[:], in0=tkg[:, 0:1], in1=tkg[:, 1:2], op=OP.subtract
            )
            gvf = sb.tile([1, E], bf16, tag="gvf")
            tm1 = sb.tile([1, E], bf16, tag="tm1")
            nc.vector.tensor_scalar(gvf[:], m2f[:], tkg[:, 1:2], None, OP.mult)
            nc.vector.tensor_scalar(tm1[:], m1f[:], dt01[:], None, OP.mult)
            nc.vector.tensor_tensor(out=gvf[:], in0=gvf[:], in1=tm1[:], op=OP.add)

            # plain one-hot rows for the prefetch injection: oh0 = m1f,
            # oh1 = m2f - m1f; transposed to [E,1] via K=1 matmuls into two
            # spare columns of the hpre PSUM tile (same 16-partition range).
            oh1f = sb.tile([1, E], bf16, tag="oh1f")
            nc.vector.tensor_tensor(
                out=oh1f[:], in0=m2f[:], in1=m1f[:], op=OP.subtract
            )
            ohT = [
                sb.tile([E, 1], bf16, tag=f"ohT{k}", name=f"ohT{k}")
                for k in range(2)
            ]

            # ----- transposes on PE: gvec [16,1], tkg broadcast [128,2] ---
            gvT = ps.tile([E, 1], f32, tag="gvT")
            nc.tensor.matmul(
                out=gvT[:],
                lhsT=gvf[:],
                rhs=pk[0:1, COL_ONE : COL_ONE + 1],
                start=True,
                stop=True,
            )
            gv_sb = sb.tile([E, 1], bf16, tag="gv_sb")
            nc.vector.tensor_copy(gv_sb[:], gvT[:])
            tkr = ps.tile([P, 2], f32, tag="tkr")
            nc.tensor.matmul(
                out=tkr[:],
                lhsT=b2pk[0:1, COL_ONES : COL_ONES + P],
                rhs=tkgb[:],
                start=True,
                stop=True,
            )
            tkgrep = sb.tile([P, 2], f32, tag="tkgrep")
            nc.vector.tensor_copy(tkgrep[:], tkr[:])

            # ----- b2 injection opens the output accumulation -------------
            eo = ps.tile([P, OC], f32, tag="eo")
            for oc in range(OC):
                nc.tensor.matmul(
                    out=eo[:, oc : oc + 1],
                    lhsT=b2pk[:, oc * P : (oc + 1) * P],
                    rhs=gv_sb[:],
                    start=True,
                    stop=False,
                )

            # ----- all-expert partial layer 1 on the prefetched chunk -----
            # hpre[e, r] = sum_{d<128} W1[e, r0+r, d] * x[d] for every expert
            # (cols PRER, PRER+1 hold the transposed one-hot columns)
            hpre = ps.tile([E, PRER + 2], f32, tag="hpre")
            for k, ohrow in enumerate([m1f, oh1f]):
                nc.tensor.matmul(
                    out=hpre[:, PRER + k : PRER + k + 1],
                    lhsT=ohrow[:],
                    rhs=pk[0:1, COL_ONE : COL_ONE + 1],
                    start=True,
                    stop=True,
                )
            for k in range(2):
                nc.vector.tensor_copy(ohT[k][:], hpre[:, PRER + k : PRER + k + 1])
            for r in range(PRER):
                nc.tensor.matmul(
                    out=hpre[:, r : r + 1],
                    lhsT=w1pr[:, r * E : (r + 1) * E],
                    rhs=pk[:, 0:1],
                    start=True,
                    stop=True,
                )
            hpsb = sb.tile([E, PRER], bf16, tag="hpsb")
            nc.vector.tensor_copy(hpsb[:], hpre[:, 0:PRER])

            # ----- layer 1 + tanh(+b1) + gate scale -----------------------
            # Column rc accumulates: the injected prefetch partial (one-hot
            # matmul over e) + the dynamically-loaded remaining d-chunks.
            hp = [
                ps.tile([P, NCH], f32, tag=f"hp{k}", name=f"hp{k}")
                for k in range(2)
            ]
            hs = [
                sb.tile([P, NCH], bf16, tag=f"hs{k}", name=f"hs{k}")
                for k in range(2)
            ]
            b1f = [
                sb.tile([P, NCH], f32, tag=f"b1f{k}", name=f"b1f{k}")
                for k in range(2)
            ]
            for k in range(2):
                # rc=0: rows 0..127 all prefetched for dc=0
                nc.tensor.matmul(
                    out=hp[k][:, 0:1],
                    lhsT=hpsb[:, 0:P],
                    rhs=ohT[k][:],
                    start=True,
                    stop=False,
                )
                for dc in range(1, DC):
                    nc.tensor.matmul(
                        out=hp[k][:, 0:1],
                        lhsT=w1t[k][:, dc * RS - PRER : dc * RS - PRER + P],
                        rhs=pk[:, dc : dc + 1],
                        start=False,
                        stop=(dc == DC - 1),
                    )
                # rc=1: rows 128..191 prefetched, 192..255 dynamic (dc=0)
                nc.tensor.matmul(
                    out=hp[k][0 : PRER - P, 1:2],
                    lhsT=hpsb[:, P:PRER],
                    rhs=ohT[k][:],
                    start=True,
                    stop=False,
                    skip_group_check=True,
                )
                nc.tensor.matmul(
                    out=hp[k][PRER - P : P, 1:2],
                    lhsT=w1t[k][:, 0 : RS - PRER],
                    rhs=pk[:, 0:1],
                    start=True,
                    stop=False,
                    skip_group_check=True,
                )
                for dc in range(1, DC):
                    nc.tensor.matmul(
                        out=hp[k][:, 1:2],
                        lhsT=w1t[k][
                            :, dc * RS + P - PRER : dc * RS + P - PRER + P
                        ],
                        rhs=pk[:, dc : dc + 1],
                        start=False,
                        stop=(dc == DC - 1),
                        skip_group_check=True,
                    )
                nc.vector.tensor_copy(b1f[k][:], w1t[k][:, W1DYN : W1DYN + NCH])
                for rc in range(NCH):
                    nc.scalar.activation(
                        hs[k][:, rc : rc + 1],
                        hp[k][:, rc : rc + 1],
                        AF.Tanh,
                        bias=b1f[k][:, rc : rc + 1],
                    )
                nc.vector.tensor_scalar(
                    hs[k][:], hs[k][:], tkgrep[:, k : k + 1], None, OP.mult
                )

            # ----- layer 2 into the shared output accumulation ------------
            for k in range(2):
                for oc in range(OC):
                    for ic in range(NCH):
                        nc.tensor.matmul(
                            out=eo[:, oc : oc + 1],
                            lhsT=w2_lhsT(k, oc, ic),
                            rhs=hs[k][:, ic : ic + 1],
                            start=False,
                            stop=(k == 1 and ic == NCH - 1),
                        )

            # ----- tail: one copy, one 8KB store --------------------------
            res = sb.tile([P, OC], f32, tag="res")
            nc.vector.tensor_copy(res[:], eo[:])
            nc.sync.dma_start(
                out_d.ap().rearrange("o (p oc) -> p (o oc)", p=P), res[:]
            )

    nc.compile()
    _BUILT = (nc, in_names, stage)
    return _BUILT[:2]


def make_in_maps(x, Wg, bg, W1, b1, W2, b2):
    """Host-side sharding + bf16 pre-swizzle: per-core input dicts."""
    x = np.asarray(x, np.float32).reshape(D)
    Wg = np.asarray(Wg, np.float32)
    bg = np.asarray(bg, np.float32).reshape(E)
    W1 = np.asarray(W1, np.float32)
    b1 = np.asarray(b1, np.float32)
    W2 = np.asarray(W2, np.float32)
    b2 = np.asarray(b2, np.float32)

    pack = np.zeros((P, PKW), np.float32)
    pack[:, 0:DC] = x.reshape(DC, P).T
    pack[:, DC : DC + DC * E] = (
        Wg.reshape(E, DC, P).transpose(2, 1, 0).reshape(P, DC * E)
    )
    pack[:, COL_ONE] = 1.0
    pack[0, COL_BG : COL_BG + E] = bg
    pack = np.ascontiguousarray(pack.astype(BF16))

    b2pk = np.zeros((E, B2W), np.float32)
    b2pk[:, 0:H] = b2 / NCORES
    b2pk[0, COL_ONES : COL_ONES + P] = 1.0
    b2pk = np.ascontiguousarray(b2pk.astype(BF16))

    in_maps = []
    for c in range(NCORES):
        rs = slice(c * RS, (c + 1) * RS)
        w1 = (
            W1[:, rs, :]
            .reshape(E, RS, DC, P)
            .transpose(0, 3, 2, 1)
            .reshape(E, P, DC * RS)
        )
        b1c = b1[:, rs].reshape(E, NCH, P).transpose(0, 2, 1)
        # first PRER (dc, r) columns of every expert go in the speculative
        # prefetch block; the dynamic per-expert load carries the rest
        w1pre = np.ascontiguousarray(
            W1[:, c * RS : c * RS + PRER, 0:P]
            .transpose(2, 1, 0)
            .reshape(P, PRER * E)
            .astype(BF16)
        )
        w1s = np.ascontiguousarray(
            np.concatenate([w1[:, :, PRER:], b1c], axis=2).astype(BF16)
        )
        w2s = np.ascontiguousarray(
            W2[:, :, rs]
            .reshape(E, H, NCH, P)
            .transpose(0, 3, 2, 1)
            .reshape(E, P, NCH * H)
            .astype(BF16)
        )
        in_maps.append(
            {
                "pack": pack,
                "b2pk": b2pk,
                "w1pre": w1pre,
                "w1s": w1s,
                "w2s": w2s,
            }
        )
    return in_maps


def unshard(outs):
    """outs: per-core [1, H] raw buffers (p-major [P, OC]) -> full [H]."""
    acc = np.zeros((P, OC), np.float32)
    for o in outs:
        acc += np.asarray(o, np.float32).reshape(P, OC)
    return np.ascontiguousarray(acc.T).reshape(H)


def kernel(x, Wg, bg, W1, b1, W2, b2, train=0, **_unused):
    from concourse import bass_utils

    nc, _ = _build()
    in_maps = make_in_maps(x, Wg, bg, W1, b1, W2, b2)
    res = bass_utils.run_bass_kernel_spmd(
        nc, in_maps, core_ids=list(range(NCORES))
    )
    return unshard([res.results[c]["out"] for c in range(NCORES)])


# revision 32
# speedup vs baseline: 1.0438x; 1.0438x over previous
"""Trainium2 Bass kernel: top-2 MoE routing (E=16, D=H=2048), 8 NeuronCores.

Strategy (memory-regime optimal: only the 2 selected experts' weights are
ever read from HBM, in bf16):
  * Weights are sharded across cores *within* each expert: core c owns rows
    [c*256, (c+1)*256) of every expert's W1 and the matching contraction
    slice of W2, both pre-swizzled on the host to the exact SBUF layout
    ([128, free]) and cast to bf16, so each dynamic load is one
    per-partition-contiguous 1MB DMA.
  * Every core redundantly computes the gating on-device from one packed
    (x | Wg.T | one | bg) DMA: logits via PE matvec, then top-2 value+index
    in two DVE ops (max / max_index).  The two expert indices go to SP
    registers, which drive the four dynamic weight DMAs.
  * b1 rides as two extra bf16 columns of each W1 slice; b2/8 is injected
    into the output PSUM accumulation by a [16x128]^T @ gvec matmul where
    gvec[e] = tkg_k for the selected experts (computed via a tiny K=1
    transpose matmul), so the post-stream tail is one PSUM->SBUF copy and
    one 8KB store.
  * tkg_k = sigmoid(+/-(v0-v1)) folded through tanh (single ACT table);
    the reference's +1e-6 softmax-sum regularizer is dropped (relative
    effect ~1e-5, far below the 2e-2 gate).
  * Each core's gate-weighted partial output is returned per-core; the
    host sums the 8 partials (free) and de-swizzles.
"""

import numpy as np

try:  # make concourse importable in bare environments
    import concourse.bacc  # noqa: F401
except ImportError:  # pragma: no cover
    import sys

    sys.path.insert(0, "/opt/trn_rl_repo")

import ml_dtypes

BF16 = ml_dtypes.bfloat16

E, D, H = 16, 2048, 2048
NCORES = 8
P = 128
RS = H // NCORES  # 256 rows of each expert held per core
NCH = RS // P  # 2 partition-chunks per 256 rows
DC = D // P  # 16 contraction chunks for layer 1
OC = H // P  # 16 output chunks for layer 2

PRER = 192  # rows of every expert's W1 (dc=0) prefetched speculatively
W1DYN = DC * RS - PRER  # 3904: dynamically-loaded W1 columns
W1W = W1DYN + NCH  # 3906: + b1 slice folded as 2 extra cols
W2W = NCH * H  # 4096
W2E1_SPLIT = True  # split W2e1 into oc0-13/oc14-15 loads (see comment below)
PKW = DC + DC * E + 1 + E  # 289: x | Wg.T | one | bg
COL_ONE = DC + DC * E  # 272
COL_BG = COL_ONE + 1  # 273
B2W = H + P  # 2176: b2/8 | ones_row (partition 0)
OPAD = 64  # f32 elements per padded output row (256B scatter payload)
COL_ONES = H  # 2048

_BUILT = None


def _build(stage=2):
    """Build + compile the Bass program once. stage kept for test.py compat."""
    global _BUILT
    if _BUILT is not None and _BUILT[2] == stage:
        return _BUILT[:2]

    import concourse.bacc as bacc
    import concourse.bass as bass
    import concourse.tile as tile
    from concourse import mybir

    f32 = mybir.dt.float32
    bf16 = mybir.dt.bfloat16
    u32 = mybir.dt.uint32
    OP = mybir.AluOpType
    AF = mybir.ActivationFunctionType

    nc = bacc.Bacc(
        "TRN2", target_bir_lowering=False, debug=False, num_devices=NCORES
    )

    # ----- I/O ------------------------------------------------------------
    pack_d = nc.dram_tensor("pack", [P, PKW], bf16, kind="ExternalInput")
    b2pk_d = nc.dram_tensor("b2pk", [E, B2W], bf16, kind="ExternalInput")
    w1pre_d = nc.dram_tensor("w1pre", [P, PRER * E], bf16, kind="ExternalInput")
    w1s_d = nc.dram_tensor("w1s", [E, P, W1W], bf16, kind="ExternalInput")
    w2s_d = nc.dram_tensor("w2s", [E, P, W2W], bf16, kind="ExternalInput")
    sidx_d = nc.dram_tensor("sidx", [P, P // 16], mybir.dt.int16, kind="ExternalInput")
    # out rows padded to 64 f32 so the scatter-add payload/stride is 256B
    out_d = nc.dram_tensor("out", [1, OPAD * P], f32, kind="ExternalOutput")

    in_names = ["pack", "b2pk", "w1pre", "w1s", "w2s", "sidx"]

    with tile.TileContext(nc) as tc:
        with (
            tc.tile_pool(name="sb", bufs=1) as sb,
            tc.tile_pool(name="ps", bufs=1, space="PSUM") as ps,
        ):
            # ----- static loads -------------------------------------------
            pk = sb.tile([P, PKW], bf16, tag="pk")
            nc.sync.dma_start(pk[:], pack_d.ap())
            # speculative prefetch: first PRER rows (dc=0) of every expert's
            # W1, streamed during the otherwise-idle gating window.  Layout
            # [p, r*E+e] so PE can matvec all 16 experts' partial h.
            w1pr = sb.tile([P, PRER * E], bf16, tag="w1pr")
            nc.sync.dma_start(w1pr[:], w1pre_d.ap())
            # b2pk rides the SP ring after the prefetch so it cannot steal
            # the prefetch's HWDGE slot (its data isn't needed until ~6us)
            b2pk = sb.tile([E, B2W], bf16, tag="b2pk")
            nc.sync.dma_start(b2pk[:], b2pk_d.ap())
            sidx = sb.tile([P, P // 16], mybir.dt.int16, tag="sidx")
            nc.sync.dma_start(sidx[:], sidx_d.ap())

            # ----- gating on PE: logits = Wg @ x (+ bg) -------------------
            lg = ps.tile([1, E], f32, tag="lg")
            for dc in range(DC):
                nc.tensor.matmul(
                    out=lg[:],
                    lhsT=pk[:, dc : dc + 1],
                    rhs=pk[:, DC + dc * E : DC + (dc + 1) * E],
                    start=(dc == 0),
                    stop=False,
                )
            nc.tensor.matmul(
                out=lg[:],
                lhsT=pk[0:1, COL_ONE : COL_ONE + 1],
                rhs=pk[0:1, COL_BG : COL_BG + E],
                start=False,
                stop=True,
            )

            # ----- top-2 value+index in two DVE ops -----------------------
            vals8 = sb.tile([1, 8], f32, tag="vals8")
            idx8 = sb.tile([1, 8], u32, tag="idx8")
            nc.vector.max(vals8[:], lg[:])
            nc.vector.max_index(idx8[:], vals8[:], lg[:])

            # ----- expert indices -> SP registers -> weight DMAs ----------
            # W2 of the 2nd expert is split so the final transfer is small
            # (oc 14..15 only): most layer-2 matmuls clear before the last
            # bytes land, shrinking the post-stream tail.
            OSP = OC - 2  # oc split point
            sv = []
            w1t = [
                sb.tile([P, W1W], bf16, tag=f"w1t{k}", name=f"w1t{k}")
                for k in range(2)
            ]
            w2t0 = sb.tile([P, W2W], bf16, tag="w2t0")
            if W2E1_SPLIT:
                w2t1a = sb.tile([P, NCH * OSP * P], bf16, tag="w2t1a")
                w2t1b = sb.tile([P, NCH * (OC - OSP) * P], bf16, tag="w2t1b")
            else:
                w2t1 = sb.tile([P, W2W], bf16, tag="w2t1")
            for k in range(2):
                reg = nc.sync.alloc_register(f"idx_sp{k}")
                nc.sync.reg_load(reg, idx8[0:1, k : k + 1])
                sv.append(nc.snap(reg, donate=True, min_val=0, max_val=E - 1))
                nc.sync.dma_start(
                    w1t[k][:],
                    w1s_d.ap()[bass.ds(sv[k], 1), :, :].rearrange(
                        "a p f -> p (a f)"
                    ),
                )
            nc.sync.dma_start(
                w2t0[:],
                w2s_d.ap()[bass.ds(sv[0], 1), :, :].rearrange("a p f -> p (a f)"),
            )
            w2e1 = w2s_d.ap()[bass.ds(sv[1], 1), :, :].rearrange(
                "a p (ic o) -> p (a ic) o", ic=NCH
            )
            if W2E1_SPLIT:
                nc.sync.dma_start(
                    w2t1a[:].rearrange("p (ic o) -> p ic o", ic=NCH),
                    w2e1[:, :, 0 : OSP * P],
                )
                nc.sync.dma_start(
                    w2t1b[:].rearrange("p (ic o) -> p ic o", ic=NCH),
                    w2e1[:, :, OSP * P : H],
                )
            else:
                # single W2e1 load keeps the HWDGE-DMA count at 8, so the
                # out-DMA's completion lane is checked LAST by the exit
                # drain (lane order = allocation order, 8 lanes round-robin)
                nc.sync.dma_start(
                    w2t1[:],
                    w2s_d.ap()[bass.ds(sv[1], 1), :, :].rearrange(
                        "a p f -> p (a f)"
                    ),
                )

            def w2_lhsT(k, oc, ic):
                if k == 0:
                    return w2t0[:, ic * H + oc * P : ic * H + (oc + 1) * P]
                if not W2E1_SPLIT:
                    return w2t1[:, ic * H + oc * P : ic * H + (oc + 1) * P]
                if oc < OSP:
                    w = OSP * P
                    return w2t1a[:, ic * w + oc * P : ic * w + (oc + 1) * P]
                w = (OC - OSP) * P
                o = oc - OSP
                return w2t1b[:, ic * w + o * P : ic * w + (o + 1) * P]

            # ----- normalized top-2 gates: tkg1 = sigmoid(v1-v0) ----------
            # sigma(z) = 0.5*tanh(z/2) + 0.5 keeps everything on the tanh
            # ACT table (no exp table load on the critical path).
            d01 = sb.tile([1, 1], f32, tag="d01")
            nc.vector.tensor_tensor(
                out=d01[:], in0=vals8[:, 1:2], in1=vals8[:, 0:1], op=OP.subtract
            )
            th01 = sb.tile([1, 1], f32, tag="th01")
            nc.scalar.activation(th01[:], d01[:], AF.Tanh, scale=0.5)
            tkg = sb.tile([1, 2], f32, tag="tkg")
            nc.vector.tensor_scalar(tkg[:, 1:2], th01[:], 0.5, 0.5, OP.mult, OP.add)
            nc.vector.tensor_scalar(
                tkg[:, 0:1], th01[:], -0.5, 0.5, OP.mult, OP.add
            )
            tkgb = sb.tile([1, 2], bf16, tag="tkgb")
            nc.vector.tensor_copy(tkgb[:], tkg[:])

            # gvf[e] = tkg_k if e == e_k else 0, in free layout:
            #   m2 = (lg >= v1) marks both, m1 = (lg >= v0) marks top-1
            #   gvf = tkg1*m2 + (tkg0-tkg1)*m1
            m1f = sb.tile([1, E], bf16, tag="m1f")
            m2f = sb.tile([1, E], bf16, tag="m2f")
            nc.vector.tensor_scalar(m2f[:], lg[:], vals8[:, 1:2], None, OP.is_ge)
            nc.vector.tensor_scalar(m1f[:], lg[:], vals8[:, 0:1], None, OP.is_ge)
            dt01 = sb.tile([1, 1], f32, tag="dt01")
            nc.vector.tensor_tensor(
                out=dt01[:], in0=tkg[:, 0:1], in1=tkg[:, 1:2], op=OP.subtract
            )
            gvf = sb.tile([1, E], bf16, tag="gvf")
            tm1 = sb.tile([1, E], bf16, tag="tm1")
            nc.vector.tensor_scalar(gvf[:], m2f[:], tkg[:, 1:2], None, OP.mult)
            nc.vector.tensor_scalar(tm1[:], m1f[:], dt01[:], None, OP.mult)
            nc.vector.tensor_tensor(out=gvf[:], in0=gvf[:], in1=tm1[:], op=OP.add)

            # plain one-hot rows for the prefetch injection: oh0 = m1f,
            # oh1 = m2f - m1f; transposed to [E,1] via K=1 matmuls into two
            # spare columns of the hpre PSUM tile (same 16-partition range).
            oh1f = sb.tile([1, E], bf16, tag="oh1f")
            nc.vector.tensor_tensor(
                out=oh1f[:], in0=m2f[:], in1=m1f[:], op=OP.subtract
            )
            ohT = [
                sb.tile([E, 1], bf16, tag=f"ohT{k}", name=f"ohT{k}")
                for k in range(2)
            ]

            # ----- transposes on PE: gvec [16,1], tkg broadcast [128,2] ---
            gvT = ps.tile([E, 1], f32, tag="gvT")
            nc.tensor.matmul(
                out=gvT[:],
                lhsT=gvf[:],
                rhs=pk[0:1, COL_ONE : COL_ONE + 1],
                start=True,
                stop=True,
            )
            gv_sb = sb.tile([E, 1], bf16, tag="gv_sb")
            nc.vector.tensor_copy(gv_sb[:], gvT[:])
            tkr = ps.tile([P, 2], f32, tag="tkr")
            nc.tensor.matmul(
                out=tkr[:],
                lhsT=b2pk[0:1, COL_ONES : COL_ONES + P],
                rhs=tkgb[:],
                start=True,
                stop=True,
            )
            tkgrep = sb.tile([P, 2], f32, tag="tkgrep")
            nc.vector.tensor_copy(tkgrep[:], tkr[:])

            # ----- b2 injection opens the output accumulation -------------
            eo = ps.tile([P, OC], f32, tag="eo")
            for oc in range(OC):
                nc.tensor.matmul(
                    out=eo[:, oc : oc + 1],
                    lhsT=b2pk[:, oc * P : (oc + 1) * P],
                    rhs=gv_sb[:],
                    start=True,
                    stop=False,
                )

            # ----- all-expert partial layer 1 on the prefetched chunk -----
            # hpre[e, r] = sum_{d<128} W1[e, r0+r, d] * x[d] for every expert
            # (cols PRER, PRER+1 hold the transposed one-hot columns)
            hpre = ps.tile([E, PRER + 2], f32, tag="hpre")
            for k, ohrow in enumerate([m1f, oh1f]):
                nc.tensor.matmul(
                    out=hpre[:, PRER + k : PRER + k + 1],
                    lhsT=ohrow[:],
                    rhs=pk[0:1, COL_ONE : COL_ONE + 1],
                    start=True,
                    stop=True,
                )
            for k in range(2):
                nc.vector.tensor_copy(ohT[k][:], hpre[:, PRER + k : PRER + k + 1])
            for r in range(PRER):
                nc.tensor.matmul(
                    out=hpre[:, r : r + 1],
                    lhsT=w1pr[:, r * E : (r + 1) * E],
                    rhs=pk[:, 0:1],
                    start=True,
                    stop=True,
                )
            hpsb = sb.tile([E, PRER], bf16, tag="hpsb")
            nc.vector.tensor_copy(hpsb[:], hpre[:, 0:PRER])

            # ----- layer 1 + tanh(+b1) + gate scale -----------------------
            # Column rc accumulates: the injected prefetch partial (one-hot
            # matmul over e) + the dynamically-loaded remaining d-chunks.
            hp = [
                ps.tile([P, NCH], f32, tag=f"hp{k}", name=f"hp{k}")
                for k in range(2)
            ]
            hs = [
                sb.tile([P, NCH], bf16, tag=f"hs{k}", name=f"hs{k}")
                for k in range(2)
            ]
            b1f = [
                sb.tile([P, NCH], f32, tag=f"b1f{k}", name=f"b1f{k}")
                for k in range(2)
            ]
            for k in range(2):
                # rc=0: rows 0..127 all prefetched for dc=0
                nc.tensor.matmul(
                    out=hp[k][:, 0:1],
                    lhsT=hpsb[:, 0:P],
                    rhs=ohT[k][:],
                    start=True,
                    stop=False,
                )
                for dc in range(1, DC):
                    nc.tensor.matmul(
                        out=hp[k][:, 0:1],
                        lhsT=w1t[k][:, dc * RS - PRER : dc * RS - PRER + P],
                        rhs=pk[:, dc : dc + 1],
                        start=False,
                        stop=(dc == DC - 1),
                    )
                # rc=1: rows 128..191 prefetched, 192..255 dynamic (dc=0)
                nc.tensor.matmul(
                    out=hp[k][0 : PRER - P, 1:2],
                    lhsT=hpsb[:, P:PRER],
                    rhs=ohT[k][:],
                    start=True,
                    stop=False,
                    skip_group_check=True,
                )
                nc.tensor.matmul(
                    out=hp[k][PRER - P : P, 1:2],
                    lhsT=w1t[k][:, 0 : RS - PRER],
                    rhs=pk[:, 0:1],
                    start=True,
                    stop=False,
                    skip_group_check=True,
                )
                for dc in range(1, DC):
                    nc.tensor.matmul(
                        out=hp[k][:, 1:2],
                        lhsT=w1t[k][
                            :, dc * RS + P - PRER : dc * RS + P - PRER + P
                        ],
                        rhs=pk[:, dc : dc + 1],
                        start=False,
                        stop=(dc == DC - 1),
                        skip_group_check=True,
                    )
                nc.vector.tensor_copy(b1f[k][:], w1t[k][:, W1DYN : W1DYN + NCH])
                for rc in range(NCH):
                    nc.scalar.activation(
                        hs[k][:, rc : rc + 1],
                        hp[k][:, rc : rc + 1],
                        AF.Tanh,
                        bias=b1f[k][:, rc : rc + 1],
                    )
                nc.vector.tensor_scalar(
                    hs[k][:], hs[k][:], tkgrep[:, k : k + 1], None, OP.mult
                )

            # ----- layer 2 into the shared output accumulation ------------
            for k in range(2):
                for oc in range(OC):
                    for ic in range(NCH):
                        nc.tensor.matmul(
                            out=eo[:, oc : oc + 1],
                            lhsT=w2_lhsT(k, oc, ic),
                            rhs=hs[k][:, ic : ic + 1],
                            start=False,
                            stop=(k == 1 and ic == NCH - 1),
                        )

            # ----- tail: one copy, one pre-built scatter store ------------
            # The store's SWDGE descriptors are built mid-stream
            # (prepare_only); after the res copy, trigger_dma fires them with
            # no HWDGE/DGE-delay latency on the critical path.  The user sem
            # is stripped from the prep so Tile pass-2 owns the completion
            # slot with its DMASW lane sem (waited on by the exit drain).
            # Scatter payloads must be 256B per index, so res rides in a
            # padded [P, OPAD] tile (cols OC..OPAD land in padding the host
            # ignores); res padding and the output region are zeroed since
            # scatter is an ADD.
            res = sb.tile([P, OPAD], f32, tag="res")
            nc.gpsimd.memset(res[:], 0.0)
            zout = sb.tile([P, OPAD], f32, tag="zout")
            nc.gpsimd.memset(zout[:], 0.0)
            nc.sync.dma_start(
                out_d.ap().rearrange("o (i e) -> (o i) e", e=OPAD), zout[:]
            )
            out_sem = nc.alloc_semaphore("out_dma_sem")
            prep = nc.gpsimd.dma_scatter_add(
                out_ap=out_d.ap().rearrange("o (i e) -> (o i) e", e=OPAD),
                in_ap=res[:].rearrange("p (a e) -> p a e", a=1),
                idxs_ap=sidx[:],
                num_idxs=P,
                num_idxs_reg=P,
                elem_size=OPAD,
                elem_step=OPAD,
                prepare_only=True,
                sem=out_sem,
            )
            _prep_inst = prep.ins if hasattr(prep, "ins") else prep
            _si = _prep_inst.sync_info
            _si.on_update = [
                u for u in _si.on_update if getattr(u, "id", None) != out_sem.num
            ]
            nc.vector.tensor_copy(res[:, 0:OC], eo[:])
            nc.gpsimd.trigger_dma(count=None)

    nc.compile()
    _BUILT = (nc, in_names, stage)
    return _BUILT[:2]


def make_in_maps(x, Wg, bg, W1, b1, W2, b2):
    """Host-side sharding + bf16 pre-swizzle: per-core input dicts."""
    x = np.asarray(x, np.float32).reshape(D)
    Wg = np.asarray(Wg, np.float32)
    bg = np.asarray(bg, np.float32).reshape(E)
    W1 = np.asarray(W1, np.float32)
    b1 = np.asarray(b1, np.float32)
    W2 = np.asarray(W2, np.float32)
    b2 = np.asarray(b2, np.float32)

    pack = np.zeros((P, PKW), np.float32)
    pack[:, 0:DC] = x.reshape(DC, P).T
    pack[:, DC : DC + DC * E] = (
        Wg.reshape(E, DC, P).transpose(2, 1, 0).reshape(P, DC * E)
    )
    pack[:, COL_ONE] = 1.0
    pack[0, COL_BG : COL_BG + E] = bg
    pack = np.ascontiguousarray(pack.astype(BF16))

    b2pk = np.zeros((E, B2W), np.float32)
    b2pk[:, 0:H] = b2 / NCORES
    b2pk[0, COL_ONES : COL_ONES + P] = 1.0
    b2pk = np.ascontiguousarray(b2pk.astype(BF16))

    # scatter row indices, wrapped in 16 partitions (idx j at [j%16, j//16]);
    # the tile spans all 128 partitions (only rows 0..15 are consumed, but
    # every value must be in-bounds), so the 16-row wrap is replicated.
    wrap16 = np.arange(P, dtype=np.int16).reshape(P // 16, 16).T
    sidx = np.ascontiguousarray(np.tile(wrap16, (P // 16, 1)))

    in_maps = []
    for c in range(NCORES):
        rs = slice(c * RS, (c + 1) * RS)
        w1 = (
            W1[:, rs, :]
            .reshape(E, RS, DC, P)
            .transpose(0, 3, 2, 1)
            .reshape(E, P, DC * RS)
        )
        b1c = b1[:, rs].reshape(E, NCH, P).transpose(0, 2, 1)
        # first PRER (dc, r) columns of every expert go in the speculative
        # prefetch block; the dynamic per-expert load carries the rest
        w1pre = np.ascontiguousarray(
            W1[:, c * RS : c * RS + PRER, 0:P]
            .transpose(2, 1, 0)
            .reshape(P, PRER * E)
            .astype(BF16)
        )
        w1s = np.ascontiguousarray(
            np.concatenate([w1[:, :, PRER:], b1c], axis=2).astype(BF16)
        )
        w2s = np.ascontiguousarray(
            W2[:, :, rs]
            .reshape(E, H, NCH, P)
            .transpose(0, 3, 2, 1)
            .reshape(E, P, NCH * H)
            .astype(BF16)
        )
        in_maps.append(
            {
                "pack": pack,
                "b2pk": b2pk,
                "w1pre": w1pre,
                "w1s": w1s,
                "w2s": w2s,
                "sidx": sidx,
            }
        )
    return in_maps


def unshard(outs):
    """outs: per-core [1, OPAD*P] padded rows (res[p] at row p) -> full [H]."""
    acc = np.zeros((P, OC), np.float32)
    for o in outs:
        acc += np.asarray(o, np.float32).reshape(P, OPAD)[:, 0:OC]
    return np.ascontiguousarray(acc.T).reshape(H)


def kernel(x, Wg, bg, W1, b1, W2, b2, train=0, **_unused):
    from concourse import bass_utils

    nc, _ = _build()
    in_maps = make_in_maps(x, Wg, bg, W1, b1, W2, b2)
    res = bass_utils.run_bass_kernel_spmd(
        nc, in_maps, core_ids=list(range(NCORES))
    )
    return unshard([res.results[c]["out"] for c in range(NCORES)])
